# revision 25
# baseline (speedup 1.0000x reference)
"""Trainium2 Bass kernel for nn_Disc_edge2 (3-layer dense-graph GNN + MLP head).

Sharding: data-parallel over batch B=16 across 8 cores (2 graphs/core).

Per-graph msg layout: [do=128 partitions, f=16384] with f = c1*2048 + t*128 + p,
edge (i, j) -> p = i, j = 8*t + c1.

All heavy compute runs as fp8e4m3 DoubleRow matmuls (2 k-tiles per pass, 0.5
cycles/row). Per 512-col chunk, layers 0/1 need just TWO DoubleRow matmuls:
    DR1: (We   @ e-chunk)   + (xib @ seli)     e-term + xi broadcast
    DR2: (xjb  @ seljm)     + (mstat @ maskA)  xj broadcast + adjacency mask
The adjacency mask is folded into the PSUM accumulation as -960*(1-A[f]) so the
relu eviction zeroes non-edges for free; no tensor-tensor mask pass exists.
Layer 2 adds a third DR for the residual e-blend (msg0@We2' + msg1@We2', with
the 0.5 folded into We2') and accumulates the edge-mean readout via accum_out
on the eviction op; msg2 is never materialized.

The j-aggregation agg@Wa runs on PE as 128 accumulating DoubleRow matmuls over
j-blocks with a two-digit fp8 decomposition of Wa (hi+lo), giving ~bf16
accuracy at fp8 speed and directly producing the transposed node update.

Evictions (PSUM->SBUF relu, the only remaining elementwise work) round-robin
across ACT / DVE / Pool in [128,1024] two-bank ops.

Weight-derived constants, selection matrices (seli/seljm/maskA) and the
transposed fp8 edge_attr are laid out host-side; the two operand "arenas" are
single SBUF tiles so DoubleRow k-tile pairs can be addressed by inserting a
[stride, 2] dim into the access patterns.
"""

import sys

sys.path.insert(0, "/opt/trn_rl_repo")

import numpy as np
import ml_dtypes

import concourse.bass as bass
from concourse import bacc
import concourse.mybir as mybir
import concourse.tile as tile

F32 = mybir.dt.float32
BF16 = mybir.dt.bfloat16
F8 = mybir.dt.float8e4
AF = mybir.ActivationFunctionType
OP = mybir.AluOpType
DR = mybir.MatmulPerfMode.DoubleRow

NPF8 = ml_dtypes.float8_e4m3
NPBF = ml_dtypes.bfloat16

B, N, DN0, DE0, DH = 16, 128, 64, 16, 128
NCORES = 8
GPC = B // NCORES
FREE = N * N              # 16384
CH = 512
NCH = FREE // CH          # 32 chunks
NPAIR = NCH // 2          # 16 chunk-pairs ([128,1024] evictions)

# ---- moving arena (fp8) column offsets ----
# ISA pattern steps are 16-bit (+-32767 elements), so each graph's L1 msg->seli
# k-tile pair needs a seli copy within 32K columns: seli (g0) + seli_b (g1).
O_SELJM = 0
O_MASKA = O_SELJM + FREE          # 16384
O_SELI = O_MASKA + FREE           # 32768
O_E0T = [O_SELI + CH, O_SELI + CH + 2048]        # per graph
O_MSG = [[O_E0T[1] + 2048, O_E0T[1] + 2048 + FREE],
         [O_E0T[1] + 2048 + 2 * FREE + CH, O_E0T[1] + 2048 + 3 * FREE + CH]]
O_SELI_B = O_MSG[0][1] + FREE     # second seli copy, just before msg0_g1
O_SCR = O_MSG[1][1] + FREE        # 2 x 1024 scratch (ACT/DVE)
MV_TOTAL = O_SCR + 2 * 1024

# ---- stationary arena (fp8) column offsets ----
# [dyn g0 | consts | dyn g1]; dyn = xib/xjb per layer
S_DYN = [0, None]
S_WE0 = 768                       # 8 x 128 block-diag variants
S_WE1 = S_WE0 + 1024
S_WE2H2 = S_WE1 + 128             # [0.5*We2 | 0.5*We2]
S_I2 = S_WE2H2 + 256              # [I | I] for j-block-sum DoubleRows
S_MSTAT = S_I2 + 256              # per graph 128
S_ZERO = S_MSTAT + 256
S_DYN[1] = S_ZERO + 128
ST_TOTAL = S_DYN[1] + 768

WEIGHT_NAMES = [
    "w_msg_0", "b_msg_0", "w_node_0", "b_node_0",
    "w_msg_1", "b_msg_1", "w_node_1", "b_node_1",
    "w_msg_2", "b_msg_2", "w_node_2", "b_node_2",
    "w_h1", "b_h1", "w_h2", "b_h2", "w_h3", "b_h3",
]

_CACHE = {}


def _pair(ap_a, ap_b):
    """AP reading ap_a as k-tile slot 0 and ap_b as slot 1 (inserted dim)."""
    c = ap_a.copy()
    c.ap.insert(1, [ap_b.offset - ap_a.offset, 2])
    return c


def build_nc():
    nc = bacc.Bacc()

    mv_d = nc.declare_dram_parameter("mvconst", [128, FREE + CH], F8,
                                     isOutput=False)
    marow_d = nc.declare_dram_parameter("marow", [GPC, FREE], F8, isOutput=False)
    st_d = nc.declare_dram_parameter("stconst", [128, S_DYN[1] - S_WE0], F8,
                                     isOutput=False)
    e0t_d = nc.declare_dram_parameter("e0t", [GPC, 128, 2048], F8, isOutput=False)
    xt_d = nc.declare_dram_parameter("xt", [GPC, DN0, 128], BF16, isOutput=False)
    wbf_d = nc.declare_dram_parameter("wbf", [128, 1280], BF16, isOutput=False)
    brep_d = nc.declare_dram_parameter("brep", [128, 384], F32, isOutput=False)
    bcol_d = nc.declare_dram_parameter("bcol", [128, 4], F32, isOutput=False)
    whd_d = nc.declare_dram_parameter("whd", [128, 257], F32, isOutput=False)
    bh3_d = nc.declare_dram_parameter("bh3", [1, 1], F32, isOutput=False)
    out_d = nc.declare_dram_parameter("out", [GPC, 1], F32, isOutput=True)

    with tile.TileContext(nc) as tc:
        import contextlib
        stack = contextlib.ExitStack()
        gbuf = stack.enter_context(tc.tile_pool(name="gbuf", bufs=1))
        small = stack.enter_context(tc.tile_pool(name="small", bufs=2))
        zpool = stack.enter_context(tc.tile_pool(name="zp", bufs=3, space="PSUM"))
        xpool = stack.enter_context(tc.tile_pool(name="xp", bufs=1, space="PSUM"))
        npool = stack.enter_context(tc.tile_pool(name="np", bufs=1, space="PSUM"))

        mva = gbuf.tile([128, MV_TOTAL], F8, tag="mva")
        sta = gbuf.tile([128, ST_TOTAL], F8, tag="sta")
        wbf = gbuf.tile([128, 1280], BF16, tag="wbf")
        brep = gbuf.tile([128, 384], F32, tag="brep")
        bcol = gbuf.tile([128, 4], F32, tag="bcol")
        whd = gbuf.tile([128, 257], F32, tag="whd")
        bh3 = gbuf.tile([1, 1], F32, tag="bh3")
        xt0 = [gbuf.tile([DN0, 128], BF16, tag=f"xt0_{g}", name=f"xt0_{g}")
               for g in range(GPC)]
        hsum = [gbuf.tile([128, NPAIR], F32, tag=f"hs{g}", name=f"hs{g}")
                for g in range(GPC)]

        # ---- loads, ordered by first consumption. dma_start costs ~1.3us of
        # SEQ issue time on SP/ACT queues but ~nothing on the Pool queue, so
        # all early-critical small loads go through Pool. ----
        nc.gpsimd.dma_start(mva[:, O_SELI:O_SELI + CH], mv_d[:, FREE:FREE + CH])
        nc.gpsimd.dma_start(xt0[0][:], xt_d[0])
        nc.gpsimd.dma_start(wbf[:], wbf_d[:, :])
        nc.gpsimd.dma_start(brep[:], brep_d[:, :])
        nc.gpsimd.dma_start(sta[:, S_WE0:S_DYN[1]], st_d[:, :])
        # maskA: memset the whole region in col pieces (f32 view for 4-byte
        # lanes), then DMA the GPC real notA rows over it
        for q in range(4):
            nc.gpsimd.memset(
                mva[:, O_MASKA + q * 4096:O_MASKA + (q + 1) * 4096]
                .bitcast(F32), 0.0)
        for g in range(GPC):
            nc.gpsimd.dma_start(mva[g:g + 1, O_MASKA:O_MASKA + FREE],
                                marow_d[g:g + 1, :])
        nc.gpsimd.dma_start(xt0[1][:], xt_d[1])
        # sync queue: seljm halves interleaved with e0t
        nc.sync.dma_start(mva[:, O_SELJM:O_SELJM + 8192], mv_d[:, 0:8192])
        nc.sync.dma_start(mva[:, O_E0T[0]:O_E0T[0] + 2048], e0t_d[0])
        nc.sync.dma_start(mva[:, O_SELJM + 8192:O_SELJM + FREE],
                          mv_d[:, 8192:FREE])
        nc.sync.dma_start(mva[:, O_E0T[1]:O_E0T[1] + 2048], e0t_d[1])
        # scalar queue: later-needed smalls
        nc.scalar.dma_start(mva[:, O_SELI_B:O_SELI_B + CH],
                            mv_d[:, FREE:FREE + CH])
        nc.scalar.dma_start(bcol[:], bcol_d[:, :])
        nc.scalar.dma_start(whd[:], whd_d[:, :])
        nc.scalar.dma_start(bh3[:], bh3_d[:, :])

        seli_ap = mva[:, O_SELI:O_SELI + CH]
        xTs = [xt0[g] for g in range(GPC)]   # updated per layer

        def msg_layer(g, l):
            """Emit chunk DRs + evictions for layer l of graph g."""
            dyn = S_DYN[g]
            xib = sta[:, dyn + l * 256:dyn + l * 256 + 128]
            xjb = sta[:, dyn + l * 256 + 128:dyn + l * 256 + 256]
            mstat = sta[:, S_MSTAT + g * 128:S_MSTAT + (g + 1) * 128]
            zero = sta[:, S_ZERO:S_ZERO + 128]
            msg_off = O_MSG[g][l] if l < 2 else None
            for cp in range(NPAIR):
                z = zpool.tile([128, 1024], F32, tag="z")
                for h in range(2):
                    k = 2 * cp + h
                    zz = z[:, h * 512:(h + 1) * 512]
                    sjm = mva[:, O_SELJM + k * CH:O_SELJM + (k + 1) * CH]
                    mka = mva[:, O_MASKA + k * CH:O_MASKA + (k + 1) * CH]
                    if l == 0:
                        c1, t4 = divmod(k, 4)
                        emv = mva[:, O_E0T[g] + t4 * CH:O_E0T[g] + (t4 + 1) * CH]
                        est = sta[:, S_WE0 + c1 * 128:S_WE0 + (c1 + 1) * 128]
                        nc.tensor.matmul(zz, _pair(est, xib), _pair(emv, seli_ap),
                                         start=True, stop=False, perf_mode=DR)
                    elif l == 1:
                        po = O_MSG[g][0]
                        emv = mva[:, po + k * CH:po + (k + 1) * CH]
                        est = sta[:, S_WE1:S_WE1 + 128]
                        sel = seli_ap if g == 0 else \
                            mva[:, O_SELI_B:O_SELI_B + CH]
                        nc.tensor.matmul(zz, _pair(est, xib), _pair(emv, sel),
                                         start=True, stop=False, perf_mode=DR)
                    else:
                        p0, p1 = O_MSG[g][0], O_MSG[g][1]
                        we2 = sta[:, S_WE2H2:S_WE2H2 + 256].rearrange(
                            "p (two m) -> p two m", two=2)
                        nc.tensor.matmul(
                            zz, we2,
                            _pair(mva[:, p0 + k * CH:p0 + (k + 1) * CH],
                                  mva[:, p1 + k * CH:p1 + (k + 1) * CH]),
                            start=True, stop=False, perf_mode=DR)
                        nc.tensor.matmul(zz, _pair(xib, xjb),
                                         _pair(seli_ap, sjm),
                                         start=False, stop=False, perf_mode=DR)
                        nc.tensor.matmul(zz, _pair(mstat, zero),
                                         _pair(mka, sjm),
                                         start=False, stop=True, perf_mode=DR)
                    if l < 2:
                        nc.tensor.matmul(zz, _pair(xjb, mstat), _pair(sjm, mka),
                                         start=False, stop=True, perf_mode=DR)
                # evict the chunk-pair: relu (+mask already in PSUM).
                # GPSIMD cannot read PSUM, so only ACT/DVE evict; ACT is
                # slightly cheaper per op, give it the odd extra (~8.5:7.5).
                on_act = cp % 2 == 0 or (cp == 15 and (g + l) % 2 == 0)
                if l < 2:
                    dst = mva[:, msg_off + cp * 1024:msg_off + (cp + 1) * 1024]
                    if on_act:
                        nc.scalar.activation(dst, z[:], AF.Relu)
                    else:
                        nc.vector.tensor_scalar(dst, z[:], 0.0, None, OP.max)
                else:
                    acc = hsum[g][:, cp:cp + 1]
                    if on_act:
                        scr = mva[:, O_SCR:O_SCR + 1024]
                        nc.scalar.activation(scr, z[:], AF.Relu, accum_out=acc)
                    else:
                        scr = mva[:, O_SCR + 1024:O_SCR + 2048]
                        nc.vector.tensor_scalar(scr, z[:], 0.0, None, OP.max,
                                                op1=OP.add, accum_out=acc)

        def xi_xj(g, l):
            """xi' = x@Wi + b, xj' = x@Wj as fp8 stationaries in the arena."""
            xT = xTs[g]
            K = DN0 if l == 0 else DH
            wcol = l * 384 if l < 2 else 768
            ps = xpool.tile([128, 256], F32, tag="xixj")
            nc.tensor.matmul(ps[:, 0:128], xT[:], wbf[0:K, wcol:wcol + 128],
                             start=True, stop=True)
            nc.tensor.matmul(ps[:, 128:256], xT[:], wbf[0:K, wcol + 128:wcol + 256],
                             start=True, stop=True)
            dyn = S_DYN[g]
            nc.vector.tensor_tensor(sta[:, dyn + l * 256:dyn + l * 256 + 128],
                                    ps[:, 0:128], brep[:, l * 128:(l + 1) * 128],
                                    op=OP.add)
            nc.vector.tensor_copy(sta[:, dyn + l * 256 + 128:dyn + l * 256 + 256],
                                  ps[:, 128:256])

        def block_sum(g, l):
            """agg[do, i] = sum_j msg block, via 64 identity-pair DoubleRows."""
            i2 = sta[:, S_I2:S_I2 + 256].rearrange("p (two m) -> p two m", two=2)
            ap = xpool.tile([128, 256], F32, tag="xixj")
            mo = O_MSG[g][l]
            for b2 in range(64):
                mb = mva[:, mo + b2 * 256:mo + (b2 + 1) * 256].rearrange(
                    "p (two m) -> p two m", two=2)
                nc.tensor.matmul(ap[:, 0:128], i2, mb, start=(b2 == 0),
                                 stop=(b2 == 63), perf_mode=DR)
            aggS = small.tile([128, 128], BF16, tag=f"agg{g}_{l}")
            nc.scalar.activation(aggS[:], ap[:, 0:128], AF.Copy)
            return aggS

        def node_update(g, l, aggS):
            """x_new^T = relu(Wx^T x^T + Wa^T agg^T + b) with bf16 Wa."""
            xT = xTs[g]
            K = DN0 if l == 0 else DH
            wcol = l * 384 + 256
            xn = npool.tile([128, 128], F32, tag="xn")
            nc.tensor.matmul(xn[:], wbf[0:K, wcol:wcol + 128], xT[:],
                             start=True, stop=False)
            nc.tensor.matmul(xn[:], wbf[:, 1024 + l * 128:1024 + (l + 1) * 128],
                             aggS[:], start=False, stop=True)
            xnT = small.tile([128, 128], BF16, tag=f"xnT{g}_{l}")
            nc.scalar.activation(xnT[:], xn[:], AF.Relu, bias=bcol[:, l:l + 1])
            if l == 1:
                xbl = small.tile([128, 128], BF16, tag=f"xbl{g}")
                nc.vector.tensor_add(xbl[:], xnT[:], xTs[g][:])
                xTs[g] = xbl
            else:
                xTs[g] = xnT

        def head(g):
            hpre = small.tile([128, 1], F32, tag=f"hp{g}")
            nc.vector.tensor_reduce(hpre[:], hsum[g][:], axis=mybir.AxisListType.X,
                                    op=OP.add)
            h1p = npool.tile([128, 128], F32, tag="xn")
            nc.tensor.matmul(h1p[:, 0:1], whd[:, 0:128], hpre[:],
                             start=True, stop=True)
            h1 = small.tile([128, 1], F32, tag=f"h1{g}")
            nc.scalar.activation(h1[:], h1p[:, 0:1], AF.Relu,
                                 bias=bcol[:, 2:3], scale=1.0 / FREE)
            h2p = npool.tile([128, 128], F32, tag="xn")
            nc.tensor.matmul(h2p[:, 0:1], whd[:, 128:256], h1[:],
                             start=True, stop=True)
            h2 = small.tile([128, 1], F32, tag=f"h2{g}")
            nc.scalar.activation(h2[:], h2p[:, 0:1], AF.Relu, bias=bcol[:, 3:4])
            h3p = npool.tile([128, 128], F32, tag="xn")
            nc.tensor.matmul(h3p[0:1, 0:1], whd[:, 256:257], h2[:],
                             start=True, stop=True)
            oval = small.tile([1, 1], F32, tag=f"ov{g}")
            nc.scalar.activation(oval[:], h3p[0:1, 0:1], AF.Identity, bias=bh3[:])
            nc.sync.dma_start(out_d[g:g + 1, :], oval[:])

        # ---- schedule: layer-interleaved across the two graphs ----
        for l in range(3):
            for g in range(GPC):
                xi_xj(g, l)
                msg_layer(g, l)
            if l < 2:
                aggs = [block_sum(g, l) for g in range(GPC)]
                for g in range(GPC):
                    node_update(g, l, aggs[g])
        for g in range(GPC):
            head(g)

        stack.close()
    nc.finalize()
    return nc


def _f8(x):
    return np.asarray(x, dtype=np.float32).astype(NPF8)


def prep_core_inputs(inputs, core):
    """Host-side layout/dtype prep for one core's GPC graphs."""
    gs = slice(core * GPC, (core + 1) * GPC)
    A = np.asarray(inputs["edge_index"][gs], np.float32)        # [GPC,N,N]
    x = np.asarray(inputs["x"][gs], np.float32)                 # [GPC,N,DN0]
    ea = np.asarray(inputs["edge_attr"][gs], np.float32)        # [GPC,N,N,DE0]
    w = {k: np.asarray(inputs[k], np.float32) for k in WEIGHT_NAMES}

    f = np.arange(FREE)
    c1f, tf, pf = f // 2048, (f // 128) % 16, f % 128
    jf = 8 * tf + c1f

    # moving constants: seljm | seli; notA rows separately
    mv = np.zeros((128, FREE + CH), NPF8)
    mv[:, 0:FREE] = (np.arange(128)[:, None] == jf[None, :]).astype(NPF8)
    seli = (np.arange(128)[:, None] == (np.arange(CH) % 128)[None, :])
    mv[:, FREE:FREE + CH] = seli.astype(NPF8)
    marow = np.zeros((GPC, FREE), NPF8)
    for g in range(GPC):
        marow[g] = (4.0 * (1.0 - A[g][pf, jf])).astype(NPF8)

    # stationary constants
    st = np.zeros((128, S_DYN[1] - S_WE0), NPF8)
    o = -S_WE0
    We0 = w["w_msg_0"][2 * DN0:]                                # [16,128]
    for c1 in range(8):
        st[16 * c1:16 * (c1 + 1), o + S_WE0 + c1 * 128:o + S_WE0 + (c1 + 1) * 128] \
            = _f8(We0)
    st[:, o + S_WE1:o + S_WE1 + 128] = _f8(w["w_msg_1"][2 * DH:])
    we2h = _f8(0.5 * w["w_msg_2"][2 * DH:])
    st[:, o + S_WE2H2:o + S_WE2H2 + 128] = we2h
    st[:, o + S_WE2H2 + 128:o + S_WE2H2 + 256] = we2h
    eye = np.eye(128, dtype=np.float32).astype(NPF8)
    st[:, o + S_I2:o + S_I2 + 128] = eye
    st[:, o + S_I2 + 128:o + S_I2 + 256] = eye
    for g in range(GPC):
        st[g, o + S_MSTAT + g * 128:o + S_MSTAT + (g + 1) * 128] = \
            np.asarray(-240.0, NPF8)

    # transposed fp8 edge features: [(j8,de), (t16,i)]
    e0t = np.ascontiguousarray(
        ea.reshape(GPC, N, 16, 8, DE0).transpose(0, 3, 4, 2, 1)
    ).reshape(GPC, 128, 2048).astype(NPF8)

    xt = np.ascontiguousarray(x.transpose(0, 2, 1)).astype(NPBF)

    # bf16 x-path weights: per layer [Wi | Wj | Wx], L2 has 0.5-folded Wi/Wj;
    # Wa per layer at 1024+
    wbf = np.zeros((128, 1280), NPBF)
    for l in range(2):
        Dn = DN0 if l == 0 else DH
        wm, wn = w[f"w_msg_{l}"], w[f"w_node_{l}"]
        wbf[0:Dn, l * 384:l * 384 + 128] = wm[0:Dn].astype(NPBF)
        wbf[0:Dn, l * 384 + 128:l * 384 + 256] = wm[Dn:2 * Dn].astype(NPBF)
        wbf[0:Dn, l * 384 + 256:l * 384 + 384] = wn[0:Dn].astype(NPBF)
        wbf[0:DH, 1024 + l * 128:1024 + (l + 1) * 128] = wn[Dn:].astype(NPBF)
    wbf[0:DH, 768:896] = (0.5 * w["w_msg_2"][0:DH]).astype(NPBF)
    wbf[0:DH, 896:1024] = (0.5 * w["w_msg_2"][DH:2 * DH]).astype(NPBF)

    brep = np.zeros((128, 384), np.float32)
    for l in range(3):
        brep[:, l * 128:(l + 1) * 128] = w[f"b_msg_{l}"][None, :]
    bcol = np.stack([w["b_node_0"], w["b_node_1"], w["b_h1"], w["b_h2"]],
                    axis=1).astype(np.float32)
    whd = np.zeros((128, 257), np.float32)
    whd[:, 0:128] = w["w_h1"]
    whd[:, 128:256] = w["w_h2"]
    whd[:, 256:257] = w["w_h3"]
    bh3 = w["b_h3"].reshape(1, 1).astype(np.float32)

    return {
        "mvconst": mv, "marow": marow, "stconst": st, "e0t": e0t, "xt": xt,
        "wbf": wbf, "brep": brep, "bcol": bcol, "whd": whd, "bh3": bh3,
    }


def kernel(**inputs):
    inputs = {k: np.asarray(v) for k, v in inputs.items()}
    if "nc" not in _CACHE:
        _CACHE["nc"] = build_nc()
    nc = _CACHE["nc"]

    in_maps = [prep_core_inputs(inputs, c) for c in range(NCORES)]

    from concourse.bass_utils import run_bass_kernel_spmd
    res = run_bass_kernel_spmd(nc, in_maps, list(range(NCORES)))
    out = np.concatenate([np.asarray(res.results[c]["out"]).reshape(-1)
                          for c in range(NCORES)])
    return out.astype(np.float32)


# revision 29
# speedup vs baseline: 1.0247x; 1.0247x over previous
"""Trainium2 Bass kernel for nn_Disc_edge2 (3-layer dense-graph GNN + MLP head).

Sharding: data-parallel over batch B=16 across 8 cores (2 graphs/core).

Per-graph msg layout: [do=128 partitions, f=16384] with f = c1*2048 + t*128 + p,
edge (i, j) -> p = i, j = 8*t + c1.

All heavy compute runs as fp8e4m3 DoubleRow matmuls (2 k-tiles per pass, 0.5
cycles/row). Per 512-col chunk, layers 0/1 need just TWO DoubleRow matmuls:
    DR1: (We   @ e-chunk)   + (xib @ seli)     e-term + xi broadcast
    DR2: (xjb  @ seljm)     + (mstat @ maskA)  xj broadcast + adjacency mask
The adjacency mask is folded into the PSUM accumulation as -960*(1-A[f]) so the
relu eviction zeroes non-edges for free; no tensor-tensor mask pass exists.
Layer 2 adds a third DR for the residual e-blend (msg0@We2' + msg1@We2', with
the 0.5 folded into We2') and accumulates the edge-mean readout via accum_out
on the eviction op; msg2 is never materialized.

The j-aggregation agg@Wa runs on PE as 128 accumulating DoubleRow matmuls over
j-blocks with a two-digit fp8 decomposition of Wa (hi+lo), giving ~bf16
accuracy at fp8 speed and directly producing the transposed node update.

Evictions (PSUM->SBUF relu, the only remaining elementwise work) round-robin
across ACT / DVE / Pool in [128,1024] two-bank ops.

Weight-derived constants, selection matrices (seli/seljm/maskA) and the
transposed fp8 edge_attr are laid out host-side; the two operand "arenas" are
single SBUF tiles so DoubleRow k-tile pairs can be addressed by inserting a
[stride, 2] dim into the access patterns.
"""

import sys

sys.path.insert(0, "/opt/trn_rl_repo")

import numpy as np
import ml_dtypes

import concourse.bass as bass
from concourse import bacc
import concourse.mybir as mybir
import concourse.tile as tile

F32 = mybir.dt.float32
BF16 = mybir.dt.bfloat16
F8 = mybir.dt.float8e4
AF = mybir.ActivationFunctionType
OP = mybir.AluOpType
DR = mybir.MatmulPerfMode.DoubleRow

NPF8 = ml_dtypes.float8_e4m3
NPBF = ml_dtypes.bfloat16

B, N, DN0, DE0, DH = 16, 128, 64, 16, 128
NCORES = 8
GPC = B // NCORES
FREE = N * N              # 16384
CH = 512
NCH = FREE // CH          # 32 chunks
NPAIR = NCH // 2          # 16 chunk-pairs ([128,1024] evictions)

# ---- moving arena (fp8) column offsets ----
# ISA pattern steps are 16-bit (+-32767 elements), so each graph's L1 msg->seli
# k-tile pair needs a seli copy within 32K columns: seli (g0) + seli_b (g1).
O_SELJM = 0
O_MASKA = O_SELJM + FREE          # 16384
O_SELI = O_MASKA + FREE           # 32768
O_E0T = [O_SELI + CH, O_SELI + CH + 2048]        # per graph
O_MSG = [[O_E0T[1] + 2048, O_E0T[1] + 2048 + FREE],
         [O_E0T[1] + 2048 + 2 * FREE + CH, O_E0T[1] + 2048 + 3 * FREE + CH]]
O_SELI_B = O_MSG[0][1] + FREE     # second seli copy, just before msg0_g1
O_SCR = O_MSG[1][1] + FREE        # 2 x 1024 scratch (ACT/DVE)
MV_TOTAL = O_SCR + 2 * 1024

# ---- stationary arena (fp8) column offsets ----
# [dyn g0 | consts | dyn g1]; dyn = xib/xjb per layer
S_DYN = [0, None]
S_WE0 = 768                       # 8 x 128 block-diag variants
S_WE1 = S_WE0 + 1024
S_WE2H2 = S_WE1 + 128             # [0.5*We2 | 0.5*We2]
S_I2 = S_WE2H2 + 256              # [I | I] for j-block-sum DoubleRows
S_MSTAT = S_I2 + 256              # per graph 128
S_ZERO = S_MSTAT + 256
S_DYN[1] = S_ZERO + 128
ST_TOTAL = S_DYN[1] + 768

WEIGHT_NAMES = [
    "w_msg_0", "b_msg_0", "w_node_0", "b_node_0",
    "w_msg_1", "b_msg_1", "w_node_1", "b_node_1",
    "w_msg_2", "b_msg_2", "w_node_2", "b_node_2",
    "w_h1", "b_h1", "w_h2", "b_h2", "w_h3", "b_h3",
]

_CACHE = {}


def _pair(ap_a, ap_b):
    """AP reading ap_a as k-tile slot 0 and ap_b as slot 1 (inserted dim)."""
    c = ap_a.copy()
    c.ap.insert(1, [ap_b.offset - ap_a.offset, 2])
    return c


def build_nc():
    nc = bacc.Bacc()

    mv_d = nc.declare_dram_parameter("mvconst", [128, FREE + CH], F8,
                                     isOutput=False)
    marow_d = nc.declare_dram_parameter("marow", [GPC, FREE], F8, isOutput=False)
    st_d = nc.declare_dram_parameter("stconst", [128, S_DYN[1] - S_WE0], F8,
                                     isOutput=False)
    e0t_d = nc.declare_dram_parameter("e0t", [GPC, 128, 2048], F8, isOutput=False)
    xt_d = nc.declare_dram_parameter("xt", [GPC, DN0, 128], BF16, isOutput=False)
    wbf_d = nc.declare_dram_parameter("wbf", [128, 1280], BF16, isOutput=False)
    brep_d = nc.declare_dram_parameter("brep", [128, 384], F32, isOutput=False)
    bcol_d = nc.declare_dram_parameter("bcol", [128, 4], F32, isOutput=False)
    whd_d = nc.declare_dram_parameter("whd", [128, 257], F32, isOutput=False)
    bh3_d = nc.declare_dram_parameter("bh3", [1, 1], F32, isOutput=False)
    out_d = nc.declare_dram_parameter("out", [GPC, 1], F32, isOutput=True)

    with tile.TileContext(nc) as tc:
        import contextlib
        stack = contextlib.ExitStack()
        gbuf = stack.enter_context(tc.tile_pool(name="gbuf", bufs=1))
        small = stack.enter_context(tc.tile_pool(name="small", bufs=2))
        zpool = stack.enter_context(tc.tile_pool(name="zp", bufs=3, space="PSUM"))
        spool = stack.enter_context(tc.tile_pool(name="sp", bufs=2, space="PSUM"))

        mva = gbuf.tile([128, MV_TOTAL], F8, tag="mva")
        sta = gbuf.tile([128, ST_TOTAL], F8, tag="sta")
        wbf = gbuf.tile([128, 1280], BF16, tag="wbf")
        brep = gbuf.tile([128, 384], F32, tag="brep")
        bcol = gbuf.tile([128, 4], F32, tag="bcol")
        whd = gbuf.tile([128, 257], F32, tag="whd")
        bh3 = gbuf.tile([1, 1], F32, tag="bh3")
        xt0 = [gbuf.tile([DN0, 128], BF16, tag=f"xt0_{g}", name=f"xt0_{g}")
               for g in range(GPC)]
        hsum = [gbuf.tile([128, NPAIR], F32, tag=f"hs{g}", name=f"hs{g}")
                for g in range(GPC)]

        # ---- loads, ordered by first consumption. dma_start costs ~1.3us of
        # SEQ issue time on SP/ACT queues but ~0.5us on the Pool queue, so
        # all early-critical small loads go through Pool. ----
        # maskA zero-fill runs split across DVE/ACT in their idle startup
        # window (f32 bitcast view for 4-byte lanes)
        for q in range(4):
            piece = mva[:, O_MASKA + q * 4096:O_MASKA + (q + 1) * 4096]
            if q < 2:
                nc.vector.memset(piece.bitcast(F32), 0.0)
            else:
                nc.scalar.memzero(piece)
        nc.gpsimd.dma_start(mva[:, O_SELI:O_SELI + CH], mv_d[:, FREE:FREE + CH])
        nc.gpsimd.dma_start(xt0[0][:], xt_d[0])
        nc.gpsimd.dma_start(wbf[:], wbf_d[:, :])
        nc.gpsimd.dma_start(brep[:], brep_d[:, :])
        nc.gpsimd.dma_start(sta[:, S_WE0:S_DYN[1]], st_d[:, :])
        for g in range(GPC):
            nc.gpsimd.dma_start(mva[g:g + 1, O_MASKA:O_MASKA + FREE],
                                marow_d[g:g + 1, :])
        nc.gpsimd.dma_start(xt0[1][:], xt_d[1])
        # sync queue: seljm halves interleaved with e0t
        nc.sync.dma_start(mva[:, O_SELJM:O_SELJM + 8192], mv_d[:, 0:8192])
        nc.sync.dma_start(mva[:, O_E0T[0]:O_E0T[0] + 2048], e0t_d[0])
        nc.sync.dma_start(mva[:, O_SELJM + 8192:O_SELJM + FREE],
                          mv_d[:, 8192:FREE])
        nc.sync.dma_start(mva[:, O_E0T[1]:O_E0T[1] + 2048], e0t_d[1])
        # scalar queue: later-needed smalls
        nc.scalar.dma_start(mva[:, O_SELI_B:O_SELI_B + CH],
                            mv_d[:, FREE:FREE + CH])
        nc.scalar.dma_start(bcol[:], bcol_d[:, :])
        nc.scalar.dma_start(whd[:], whd_d[:, :])
        nc.scalar.dma_start(bh3[:], bh3_d[:, :])

        seli_ap = mva[:, O_SELI:O_SELI + CH]
        xTs = [xt0[g] for g in range(GPC)]   # updated per layer

        def msg_layer(g, l):
            """Emit chunk DRs + evictions for layer l of graph g."""
            dyn = S_DYN[g]
            xib = sta[:, dyn + l * 256:dyn + l * 256 + 128]
            xjb = sta[:, dyn + l * 256 + 128:dyn + l * 256 + 256]
            mstat = sta[:, S_MSTAT + g * 128:S_MSTAT + (g + 1) * 128]
            zero = sta[:, S_ZERO:S_ZERO + 128]
            msg_off = O_MSG[g][l] if l < 2 else None
            for cp in range(NPAIR):
                z = zpool.tile([128, 1024], F32, tag="z")
                for h in range(2):
                    k = 2 * cp + h
                    zz = z[:, h * 512:(h + 1) * 512]
                    sjm = mva[:, O_SELJM + k * CH:O_SELJM + (k + 1) * CH]
                    mka = mva[:, O_MASKA + k * CH:O_MASKA + (k + 1) * CH]
                    if l == 0:
                        c1, t4 = divmod(k, 4)
                        emv = mva[:, O_E0T[g] + t4 * CH:O_E0T[g] + (t4 + 1) * CH]
                        est = sta[:, S_WE0 + c1 * 128:S_WE0 + (c1 + 1) * 128]
                        nc.tensor.matmul(zz, _pair(est, xib), _pair(emv, seli_ap),
                                         start=True, stop=False, perf_mode=DR)
                    elif l == 1:
                        po = O_MSG[g][0]
                        emv = mva[:, po + k * CH:po + (k + 1) * CH]
                        est = sta[:, S_WE1:S_WE1 + 128]
                        sel = seli_ap if g == 0 else \
                            mva[:, O_SELI_B:O_SELI_B + CH]
                        nc.tensor.matmul(zz, _pair(est, xib), _pair(emv, sel),
                                         start=True, stop=False, perf_mode=DR)
                    else:
                        p0, p1 = O_MSG[g][0], O_MSG[g][1]
                        we2 = sta[:, S_WE2H2:S_WE2H2 + 256].rearrange(
                            "p (two m) -> p two m", two=2)
                        nc.tensor.matmul(
                            zz, we2,
                            _pair(mva[:, p0 + k * CH:p0 + (k + 1) * CH],
                                  mva[:, p1 + k * CH:p1 + (k + 1) * CH]),
                            start=True, stop=False, perf_mode=DR)
                        nc.tensor.matmul(zz, _pair(xib, xjb),
                                         _pair(seli_ap, sjm),
                                         start=False, stop=False, perf_mode=DR)
                        nc.tensor.matmul(zz, _pair(mstat, zero),
                                         _pair(mka, sjm),
                                         start=False, stop=True, perf_mode=DR)
                    if l < 2:
                        nc.tensor.matmul(zz, _pair(xjb, mstat), _pair(sjm, mka),
                                         start=False, stop=True, perf_mode=DR)
                # evict the chunk-pair: relu (+mask already in PSUM).
                # GPSIMD cannot read PSUM, so only ACT/DVE evict; ACT is
                # slightly cheaper per op, give it the odd extra (~8.5:7.5).
                on_act = cp % 2 == 0 or (cp == 15 and (g + l) % 2 == 0)
                if l < 2:
                    dst = mva[:, msg_off + cp * 1024:msg_off + (cp + 1) * 1024]
                    if on_act:
                        nc.scalar.activation(dst, z[:], AF.Relu)
                    else:
                        nc.vector.tensor_scalar(dst, z[:], 0.0, None, OP.max)
                else:
                    acc = hsum[g][:, cp:cp + 1]
                    if on_act:
                        scr = mva[:, O_SCR:O_SCR + 1024]
                        nc.scalar.activation(scr, z[:], AF.Relu, accum_out=acc)
                    else:
                        scr = mva[:, O_SCR + 1024:O_SCR + 2048]
                        nc.vector.tensor_scalar(scr, z[:], 0.0, None, OP.max,
                                                op1=OP.add, accum_out=acc)

        def xi_xj(g, l):
            """xi' = x@Wi + b, xj' = x@Wj as fp8 stationaries in the arena."""
            xT = xTs[g]
            K = DN0 if l == 0 else DH
            wcol = l * 384 if l < 2 else 768
            ps = spool.tile([128, 512], F32, tag="sp")
            nc.tensor.matmul(ps[:, 0:128], xT[:], wbf[0:K, wcol:wcol + 128],
                             start=True, stop=True)
            nc.tensor.matmul(ps[:, 128:256], xT[:], wbf[0:K, wcol + 128:wcol + 256],
                             start=True, stop=True)
            dyn = S_DYN[g]
            nc.vector.tensor_tensor(sta[:, dyn + l * 256:dyn + l * 256 + 128],
                                    ps[:, 0:128], brep[:, l * 128:(l + 1) * 128],
                                    op=OP.add)
            nc.vector.tensor_copy(sta[:, dyn + l * 256 + 128:dyn + l * 256 + 256],
                                  ps[:, 128:256])

        def block_sum(g, l):
            """agg[do, i] = sum_j msg block, via 64 identity-pair DoubleRows."""
            i2 = sta[:, S_I2:S_I2 + 256].rearrange("p (two m) -> p two m", two=2)
            ap = spool.tile([128, 512], F32, tag="sp")
            mo = O_MSG[g][l]
            for b2 in range(64):
                mb = mva[:, mo + b2 * 256:mo + (b2 + 1) * 256].rearrange(
                    "p (two m) -> p two m", two=2)
                nc.tensor.matmul(ap[:, 0:128], i2, mb, start=(b2 == 0),
                                 stop=(b2 == 63), perf_mode=DR)
            aggS = small.tile([128, 128], BF16, tag=f"agg{g}_{l}")
            nc.vector.tensor_copy(aggS[:], ap[:, 0:128])
            return aggS

        def node_update(g, l, aggS):
            """x_new^T = relu(Wx^T x^T + Wa^T agg^T + b) with bf16 Wa."""
            xT = xTs[g]
            K = DN0 if l == 0 else DH
            wcol = l * 384 + 256
            xnt_ = spool.tile([128, 512], F32, tag="sp")
            xn = xnt_[:, 0:128]
            nc.tensor.matmul(xn, wbf[0:K, wcol:wcol + 128], xT[:],
                             start=True, stop=False)
            nc.tensor.matmul(xn, wbf[:, 1024 + l * 128:1024 + (l + 1) * 128],
                             aggS[:], start=False, stop=True)
            xnT = small.tile([128, 128], BF16, tag=f"xnT{g}_{l}")
            nc.scalar.activation(xnT[:], xn, AF.Relu, bias=bcol[:, l:l + 1])
            if l == 1:
                xbl = small.tile([128, 128], BF16, tag=f"xbl{g}")
                nc.vector.tensor_add(xbl[:], xnT[:], xTs[g][:])
                xTs[g] = xbl
            else:
                xTs[g] = xnT

        def head(g):
            hpre = small.tile([128, 1], F32, tag=f"hp{g}")
            nc.vector.tensor_reduce(hpre[:], hsum[g][:], axis=mybir.AxisListType.X,
                                    op=OP.add)
            hp_ = spool.tile([128, 512], F32, tag="sp")
            h1p = hp_[:, 0:128]
            nc.tensor.matmul(h1p[:, 0:1], whd[:, 0:128], hpre[:],
                             start=True, stop=True)
            h1 = small.tile([128, 1], F32, tag=f"h1{g}")
            nc.scalar.activation(h1[:], h1p[:, 0:1], AF.Relu,
                                 bias=bcol[:, 2:3], scale=1.0 / FREE)
            hp2_ = spool.tile([128, 512], F32, tag="sp")
            h2p = hp2_[:, 0:128]
            nc.tensor.matmul(h2p[:, 0:1], whd[:, 128:256], h1[:],
                             start=True, stop=True)
            h2 = small.tile([128, 1], F32, tag=f"h2{g}")
            nc.scalar.activation(h2[:], h2p[:, 0:1], AF.Relu, bias=bcol[:, 3:4])
            hp3_ = spool.tile([128, 512], F32, tag="sp")
            h3p = hp3_[:, 0:128]
            nc.tensor.matmul(h3p[0:1, 0:1], whd[:, 256:257], h2[:],
                             start=True, stop=True)
            oval = small.tile([1, 1], F32, tag=f"ov{g}")
            nc.scalar.activation(oval[:], h3p[0:1, 0:1], AF.Identity, bias=bh3[:])
            nc.sync.dma_start(out_d[g:g + 1, :], oval[:])

        # ---- schedule: layer-interleaved across the two graphs ----
        for l in range(3):
            for g in range(GPC):
                xi_xj(g, l)
                msg_layer(g, l)
            if l < 2:
                aggs = [block_sum(g, l) for g in range(GPC)]
                for g in range(GPC):
                    node_update(g, l, aggs[g])
        for g in range(GPC):
            head(g)

        stack.close()
    nc.finalize()
    return nc


def _f8(x):
    return np.asarray(x, dtype=np.float32).astype(NPF8)


def prep_core_inputs(inputs, core):
    """Host-side layout/dtype prep for one core's GPC graphs."""
    gs = slice(core * GPC, (core + 1) * GPC)
    A = np.asarray(inputs["edge_index"][gs], np.float32)        # [GPC,N,N]
    x = np.asarray(inputs["x"][gs], np.float32)                 # [GPC,N,DN0]
    ea = np.asarray(inputs["edge_attr"][gs], np.float32)        # [GPC,N,N,DE0]
    w = {k: np.asarray(inputs[k], np.float32) for k in WEIGHT_NAMES}

    f = np.arange(FREE)
    c1f, tf, pf = f // 2048, (f // 128) % 16, f % 128
    jf = 8 * tf + c1f

    # moving constants: seljm | seli; notA rows separately
    mv = np.zeros((128, FREE + CH), NPF8)
    mv[:, 0:FREE] = (np.arange(128)[:, None] == jf[None, :]).astype(NPF8)
    seli = (np.arange(128)[:, None] == (np.arange(CH) % 128)[None, :])
    mv[:, FREE:FREE + CH] = seli.astype(NPF8)
    marow = np.zeros((GPC, FREE), NPF8)
    for g in range(GPC):
        marow[g] = (4.0 * (1.0 - A[g][pf, jf])).astype(NPF8)

    # stationary constants
    st = np.zeros((128, S_DYN[1] - S_WE0), NPF8)
    o = -S_WE0
    We0 = w["w_msg_0"][2 * DN0:]                                # [16,128]
    for c1 in range(8):
        st[16 * c1:16 * (c1 + 1), o + S_WE0 + c1 * 128:o + S_WE0 + (c1 + 1) * 128] \
            = _f8(We0)
    st[:, o + S_WE1:o + S_WE1 + 128] = _f8(w["w_msg_1"][2 * DH:])
    we2h = _f8(0.5 * w["w_msg_2"][2 * DH:])
    st[:, o + S_WE2H2:o + S_WE2H2 + 128] = we2h
    st[:, o + S_WE2H2 + 128:o + S_WE2H2 + 256] = we2h
    eye = np.eye(128, dtype=np.float32).astype(NPF8)
    st[:, o + S_I2:o + S_I2 + 128] = eye
    st[:, o + S_I2 + 128:o + S_I2 + 256] = eye
    for g in range(GPC):
        st[g, o + S_MSTAT + g * 128:o + S_MSTAT + (g + 1) * 128] = \
            np.asarray(-240.0, NPF8)

    # transposed fp8 edge features: [(j8,de), (t16,i)]
    e0t = np.ascontiguousarray(
        ea.reshape(GPC, N, 16, 8, DE0).transpose(0, 3, 4, 2, 1)
    ).reshape(GPC, 128, 2048).astype(NPF8)

    xt = np.ascontiguousarray(x.transpose(0, 2, 1)).astype(NPBF)

    # bf16 x-path weights: per layer [Wi | Wj | Wx], L2 has 0.5-folded Wi/Wj;
    # Wa per layer at 1024+
    wbf = np.zeros((128, 1280), NPBF)
    for l in range(2):
        Dn = DN0 if l == 0 else DH
        wm, wn = w[f"w_msg_{l}"], w[f"w_node_{l}"]
        wbf[0:Dn, l * 384:l * 384 + 128] = wm[0:Dn].astype(NPBF)
        wbf[0:Dn, l * 384 + 128:l * 384 + 256] = wm[Dn:2 * Dn].astype(NPBF)
        wbf[0:Dn, l * 384 + 256:l * 384 + 384] = wn[0:Dn].astype(NPBF)
        wbf[0:DH, 1024 + l * 128:1024 + (l + 1) * 128] = wn[Dn:].astype(NPBF)
    wbf[0:DH, 768:896] = (0.5 * w["w_msg_2"][0:DH]).astype(NPBF)
    wbf[0:DH, 896:1024] = (0.5 * w["w_msg_2"][DH:2 * DH]).astype(NPBF)

    brep = np.zeros((128, 384), np.float32)
    for l in range(3):
        brep[:, l * 128:(l + 1) * 128] = w[f"b_msg_{l}"][None, :]
    bcol = np.stack([w["b_node_0"], w["b_node_1"], w["b_h1"], w["b_h2"]],
                    axis=1).astype(np.float32)
    whd = np.zeros((128, 257), np.float32)
    whd[:, 0:128] = w["w_h1"]
    whd[:, 128:256] = w["w_h2"]
    whd[:, 256:257] = w["w_h3"]
    bh3 = w["b_h3"].reshape(1, 1).astype(np.float32)

    return {
        "mvconst": mv, "marow": marow, "stconst": st, "e0t": e0t, "xt": xt,
        "wbf": wbf, "brep": brep, "bcol": bcol, "whd": whd, "bh3": bh3,
    }


def kernel(**inputs):
    inputs = {k: np.asarray(v) for k, v in inputs.items()}
    if "nc" not in _CACHE:
        _CACHE["nc"] = build_nc()
    nc = _CACHE["nc"]

    in_maps = [prep_core_inputs(inputs, c) for c in range(NCORES)]

    from concourse.bass_utils import run_bass_kernel_spmd
    res = run_bass_kernel_spmd(nc, in_maps, list(range(NCORES)))
    out = np.concatenate([np.asarray(res.results[c]["out"]).reshape(-1)
                          for c in range(NCORES)])
    return out.astype(np.float32)


# revision 34
# speedup vs baseline: 1.0596x; 1.0341x over previous
"""Trainium2 Bass kernel for nn_Disc_edge2 (3-layer dense-graph GNN + MLP head).

Sharding: data-parallel over batch B=16 across 8 cores (2 graphs/core).

Per-graph msg layout: [do=128 partitions, f=16384] with f = c1*2048 + t*128 + p,
edge (i, j) -> p = i, j = 8*t + c1.

All heavy compute runs as fp8e4m3 DoubleRow matmuls (2 k-tiles per pass, 0.5
cycles/row). Per 512-col chunk, layers 0/1 need just TWO DoubleRow matmuls:
    DR1: (We   @ e-chunk)   + (xib @ seli)     e-term + xi broadcast
    DR2: (xjb  @ seljm)     + (mstat @ maskA)  xj broadcast + adjacency mask
The adjacency mask is folded into the PSUM accumulation as -960*(1-A[f]) so the
relu eviction zeroes non-edges for free; no tensor-tensor mask pass exists.
Layer 2 adds a third DR for the residual e-blend (msg0@We2' + msg1@We2', with
the 0.5 folded into We2') and accumulates the edge-mean readout via accum_out
on the eviction op; msg2 is never materialized.

The j-aggregation runs on PE as accumulating [I|I] DoubleRows over j-block
pairs (block-sum in PSUM), interleaved into the chunk stream two pairs behind
the evictions; agg@Wa then happens in bf16 on the node-update matmul.

Evictions (PSUM->SBUF relu, the only remaining elementwise work) alternate
ACT / DVE in [128,1024] two-bank ops. GPSIMD cannot touch PSUM.

DMA issue costs ~1.3us of sequencer time per dma_start, so all constants are
byte-packed host-side into a handful of big transfers and sub-addressed with
bitcast views. The operand "arenas" are single SBUF tiles so DoubleRow k-tile
pairs can be addressed by inserting a [stride, 2] dim into the access
patterns (ISA steps are 16-bit, hence the second seli copy near graph 1's msg
regions).
"""

import sys

sys.path.insert(0, "/opt/trn_rl_repo")

import numpy as np
import ml_dtypes

import concourse.bass as bass
from concourse import bacc
import concourse.mybir as mybir
import concourse.tile as tile

F32 = mybir.dt.float32
BF16 = mybir.dt.bfloat16
F8 = mybir.dt.float8e4
AF = mybir.ActivationFunctionType
OP = mybir.AluOpType
DR = mybir.MatmulPerfMode.DoubleRow

NPF8 = ml_dtypes.float8_e4m3
NPBF = ml_dtypes.bfloat16

B, N, DN0, DE0, DH = 16, 128, 64, 16, 128
NCORES = 8
GPC = B // NCORES
FREE = N * N              # 16384
CH = 512
NCH = FREE // CH          # 32 chunks
NPAIR = NCH // 2          # 16 chunk-pairs ([128,1024] evictions)

# ---- moving arena (fp8) column offsets ----
O_SELJM = 0
O_MASKA = O_SELJM + FREE          # 16384
O_SELI = O_MASKA + FREE           # 32768
O_E0T = [O_SELI + CH, O_SELI + CH + 2048]        # per graph
O_MSG = [[O_E0T[1] + 2048, O_E0T[1] + 2048 + FREE],
         [O_E0T[1] + 2048 + 2 * FREE + CH, O_E0T[1] + 2048 + 3 * FREE + CH]]
O_SELI_B = O_MSG[0][1] + FREE     # second seli copy, just before msg0_g1
O_SCR = O_MSG[1][1] + FREE        # 2 x 1024 scratch (ACT/DVE)
MV_TOTAL = O_SCR + 2 * 1024

# ---- stationary arena (fp8 tile, byte-addressed constants) ----
# [dyn g0 | xt | wbf | brep | fp8 consts | seli_src | dyn g1]
S_DYN = [0, None]
S_XT = 768                        # [64,128] bf16 per graph, side by side
S_WBF = S_XT + 512                # [128,1280] bf16
S_BREP = S_WBF + 2560             # [128,384] f32
S_WE0 = S_BREP + 1536             # 8 x 128 block-diag variants
S_WE1 = S_WE0 + 1024
S_WE2H2 = S_WE1 + 128             # [0.5*We2 | 0.5*We2]
S_I2 = S_WE2H2 + 256              # [I | I] for j-block-sum DoubleRows
S_MSTAT = S_I2 + 256              # per graph 128
S_ZERO = S_MSTAT + 256
S_SELI = S_ZERO + 128             # seli content, copied into mva
S_DYN[1] = S_SELI + 512
ST_TOTAL = S_DYN[1] + 768
# pk2: bcol(16B) | whd(1028B) | bh3(4B)
PK2_TOTAL = 1048

WEIGHT_NAMES = [
    "w_msg_0", "b_msg_0", "w_node_0", "b_node_0",
    "w_msg_1", "b_msg_1", "w_node_1", "b_node_1",
    "w_msg_2", "b_msg_2", "w_node_2", "b_node_2",
    "w_h1", "b_h1", "w_h2", "b_h2", "w_h3", "b_h3",
]

_CACHE = {}


def _pair(ap_a, ap_b):
    """AP reading ap_a as k-tile slot 0 and ap_b as slot 1 (inserted dim)."""
    c = ap_a.copy()
    c.ap.insert(1, [ap_b.offset - ap_a.offset, 2])
    return c


def build_nc():
    nc = bacc.Bacc()

    stc_d = nc.declare_dram_parameter("stconst", [128, S_DYN[1] - S_XT], F8,
                                      isOutput=False)
    sj_d = nc.declare_dram_parameter("seljm", [128, FREE], F8, isOutput=False)
    marow_d = nc.declare_dram_parameter("marow", [GPC, FREE], F8, isOutput=False)
    e0t_d = nc.declare_dram_parameter("e0t", [128, GPC * 2048], F8,
                                      isOutput=False)
    pk2_d = nc.declare_dram_parameter("pk2", [128, PK2_TOTAL], F8,
                                      isOutput=False)
    out_d = nc.declare_dram_parameter("out", [GPC, 1], F32, isOutput=True)

    with tile.TileContext(nc) as tc:
        import contextlib
        stack = contextlib.ExitStack()
        gbuf = stack.enter_context(tc.tile_pool(name="gbuf", bufs=1))
        small = stack.enter_context(tc.tile_pool(name="small", bufs=2))
        zpool = stack.enter_context(tc.tile_pool(name="zp", bufs=3, space="PSUM"))
        spool = stack.enter_context(tc.tile_pool(name="sp", bufs=2, space="PSUM"))

        mva = gbuf.tile([128, MV_TOTAL], F8, tag="mva")
        sta = gbuf.tile([128, ST_TOTAL], F8, tag="sta")
        pk2 = gbuf.tile([128, PK2_TOTAL], F8, tag="pk2")
        hsum = [gbuf.tile([128, NPAIR], F32, tag=f"hs{g}", name=f"hs{g}")
                for g in range(GPC)]

        # maskA zero-fill split across DVE/ACT in their idle startup window
        for q in range(4):
            piece = mva[:, O_MASKA + q * 4096:O_MASKA + (q + 1) * 4096]
            if q < 2:
                nc.vector.memset(piece.bitcast(F32), 0.0)
            else:
                nc.scalar.memzero(piece)

        # ---- six DMAs on the sync queue, in consumption order ----
        nc.sync.dma_start(sta[:, S_XT:S_DYN[1]], stc_d[:, :])
        nc.sync.dma_start(mva[:, O_SELJM:O_SELJM + 8192], sj_d[:, 0:8192])
        nc.sync.dma_start(mva[:, O_E0T[0]:O_E0T[0] + GPC * 2048], e0t_d[:, :])
        nc.sync.dma_start(mva[0:GPC, O_MASKA:O_MASKA + FREE], marow_d[:, :])
        nc.sync.dma_start(pk2[:], pk2_d[:, :])
        nc.sync.dma_start(mva[:, O_SELJM + 8192:O_SELJM + FREE],
                          sj_d[:, 8192:FREE])

        # seli copies into the moving arena
        nc.vector.tensor_copy(mva[:, O_SELI:O_SELI + CH],
                              sta[:, S_SELI:S_SELI + 512])
        nc.vector.tensor_copy(mva[:, O_SELI_B:O_SELI_B + CH],
                              sta[:, S_SELI:S_SELI + 512])

        def wslice(p0, p1, c0, c1):            # bf16 view of wbf
            return sta[p0:p1, S_WBF + 2 * c0:S_WBF + 2 * c1].bitcast(BF16)

        def brep_ap(l):
            return sta[:, S_BREP + 512 * l:S_BREP + 512 * (l + 1)].bitcast(F32)

        bcol = pk2[:, 0:16].bitcast(F32)       # [128,4]
        whd = pk2[:, 16:1044].bitcast(F32)     # [128,257]
        bh3 = pk2[0:1, 1044:1048].bitcast(F32)

        seli_ap = mva[:, O_SELI:O_SELI + CH]
        xTs = [sta[0:DN0, S_XT:S_XT + 256].bitcast(BF16),
               sta[0:DN0, S_XT + 256:S_XT + 512].bitcast(BF16)]

        def xi_xj(g, l):
            """xi' = x@Wi + b, xj' = x@Wj as fp8 stationaries in the arena."""
            xT = xTs[g]
            K = DN0 if l == 0 else DH
            wcol = l * 384 if l < 2 else 768
            ps = zpool.tile([128, 1024], F32, tag="z")
            nc.tensor.matmul(ps[:, 0:128], xT,
                             wslice(0, K, wcol, wcol + 128),
                             start=True, stop=True)
            nc.tensor.matmul(ps[:, 128:256], xT,
                             wslice(0, K, wcol + 128, wcol + 256),
                             start=True, stop=True)
            dyn = S_DYN[g]
            nc.vector.tensor_tensor(sta[:, dyn + l * 256:dyn + l * 256 + 128],
                                    ps[:, 0:128], brep_ap(l), op=OP.add)
            nc.vector.tensor_copy(sta[:, dyn + l * 256 + 128:dyn + l * 256 + 256],
                                  ps[:, 128:256])

        i2_ap = sta[:, S_I2:S_I2 + 256].rearrange("p (two m) -> p two m", two=2)
        we2_ap = sta[:, S_WE2H2:S_WE2H2 + 256].rearrange(
            "p (two m) -> p two m", two=2)

        def msg_half(g, l, half, bs):
            """Chunk-pair DRs + evictions for half a layer; block-sum DRs
            (bs = [psum_ap, next_block]) trail the evictions by 2 pairs."""
            dyn = S_DYN[g]
            xib = sta[:, dyn + l * 256:dyn + l * 256 + 128]
            xjb = sta[:, dyn + l * 256 + 128:dyn + l * 256 + 256]
            mstat = sta[:, S_MSTAT + g * 128:S_MSTAT + (g + 1) * 128]
            zero = sta[:, S_ZERO:S_ZERO + 128]
            mo = O_MSG[g][l] if l < 2 else None

            def drain_bs(upto):
                if bs is None:
                    return
                while bs[1] < upto:
                    b2 = bs[1]
                    mb = mva[:, mo + b2 * 256:mo + (b2 + 1) * 256].rearrange(
                        "p (two m) -> p two m", two=2)
                    nc.tensor.matmul(bs[0], i2_ap, mb, start=(b2 == 0),
                                     stop=(b2 == 63), perf_mode=DR)
                    bs[1] += 1

            for cp in range(half * (NPAIR // 2), (half + 1) * (NPAIR // 2)):
                z = zpool.tile([128, 1024], F32, tag="z")
                for h in range(2):
                    k = 2 * cp + h
                    zz = z[:, h * 512:(h + 1) * 512]
                    sjm = mva[:, O_SELJM + k * CH:O_SELJM + (k + 1) * CH]
                    mka = mva[:, O_MASKA + k * CH:O_MASKA + (k + 1) * CH]
                    if l == 0:
                        c1, t4 = divmod(k, 4)
                        emv = mva[:, O_E0T[g] + t4 * CH:O_E0T[g] + (t4 + 1) * CH]
                        est = sta[:, S_WE0 + c1 * 128:S_WE0 + (c1 + 1) * 128]
                        nc.tensor.matmul(zz, _pair(est, xib), _pair(emv, seli_ap),
                                         start=True, stop=False, perf_mode=DR)
                    elif l == 1:
                        po = O_MSG[g][0]
                        emv = mva[:, po + k * CH:po + (k + 1) * CH]
                        est = sta[:, S_WE1:S_WE1 + 128]
                        sel = seli_ap if g == 0 else \
                            mva[:, O_SELI_B:O_SELI_B + CH]
                        nc.tensor.matmul(zz, _pair(est, xib), _pair(emv, sel),
                                         start=True, stop=False, perf_mode=DR)
                    else:
                        p0, p1 = O_MSG[g][0], O_MSG[g][1]
                        nc.tensor.matmul(
                            zz, we2_ap,
                            _pair(mva[:, p0 + k * CH:p0 + (k + 1) * CH],
                                  mva[:, p1 + k * CH:p1 + (k + 1) * CH]),
                            start=True, stop=False, perf_mode=DR)
                        nc.tensor.matmul(zz, _pair(xib, xjb),
                                         _pair(seli_ap, sjm),
                                         start=False, stop=False, perf_mode=DR)
                        nc.tensor.matmul(zz, _pair(mstat, zero),
                                         _pair(mka, sjm),
                                         start=False, stop=True, perf_mode=DR)
                    if l < 2:
                        nc.tensor.matmul(zz, _pair(xjb, mstat), _pair(sjm, mka),
                                         start=False, stop=True, perf_mode=DR)
                # block-sum DRs trail the evictions by one pair
                drain_bs(max(0, (cp - 1) * 4))
                # evict the chunk-pair: relu (+mask already in PSUM)
                on_act = cp % 2 == 0 or (cp == 15 and (g + l) % 2 == 0)
                if l < 2:
                    dst = mva[:, mo + cp * 1024:mo + (cp + 1) * 1024]
                    if on_act:
                        nc.scalar.activation(dst, z[:], AF.Relu)
                    else:
                        nc.vector.tensor_scalar(dst, z[:], 0.0, None, OP.max)
                else:
                    acc = hsum[g][:, cp:cp + 1]
                    if on_act:
                        scr = mva[:, O_SCR:O_SCR + 1024]
                        nc.scalar.activation(scr, z[:], AF.Relu, accum_out=acc)
                    else:
                        scr = mva[:, O_SCR + 1024:O_SCR + 2048]
                        nc.vector.tensor_scalar(scr, z[:], 0.0, None, OP.max,
                                                op1=OP.add, accum_out=acc)

        def finish_agg(g, l, bs):
            """Drain remaining block-sum DRs, evict agg to bf16 SBUF."""
            dyn = S_DYN[g]
            mo = O_MSG[g][l]
            while bs[1] < 64:
                b2 = bs[1]
                mb = mva[:, mo + b2 * 256:mo + (b2 + 1) * 256].rearrange(
                    "p (two m) -> p two m", two=2)
                nc.tensor.matmul(bs[0], i2_ap, mb, start=(b2 == 0),
                                 stop=(b2 == 63), perf_mode=DR)
                bs[1] += 1
            aggS = small.tile([128, 128], BF16, tag=f"agg{g}_{l}")
            nc.vector.tensor_copy(aggS[:], bs[0])
            return aggS

        def node_update(g, l, aggS):
            """x_new^T = relu(Wx^T x^T + Wa^T agg^T + b) with bf16 Wa."""
            xT = xTs[g]
            K = DN0 if l == 0 else DH
            wcol = l * 384 + 256
            xnt = spool.tile([128, 512], F32, tag="sp", name=f"xn{g}_{l}")
            xn = xnt[:, 0:128]
            nc.tensor.matmul(xn, wslice(0, K, wcol, wcol + 128), xT,
                             start=True, stop=False)
            nc.tensor.matmul(xn, wslice(0, DH, 1024 + l * 128,
                                        1024 + (l + 1) * 128),
                             aggS[:], start=False, stop=True)
            xnT = small.tile([128, 128], BF16, tag=f"xnT{g}_{l}")
            nc.scalar.activation(xnT[:], xn, AF.Relu, bias=bcol[:, l:l + 1])
            if l == 1:
                xbl = small.tile([128, 128], BF16, tag=f"xbl{g}")
                nc.vector.tensor_add(xbl[:], xnT[:], xTs[g])
                xTs[g] = xbl[:]
            else:
                xTs[g] = xnT[:]

        def head(g):
            hpre = small.tile([128, 1], F32, tag=f"hp{g}")
            nc.vector.tensor_reduce(hpre[:], hsum[g][:], axis=mybir.AxisListType.X,
                                    op=OP.add)
            hp_ = spool.tile([128, 512], F32, tag="sp")
            nc.tensor.matmul(hp_[:, 0:1], whd[:, 0:128], hpre[:],
                             start=True, stop=True)
            h1 = small.tile([128, 1], F32, tag=f"h1{g}")
            nc.scalar.activation(h1[:], hp_[:, 0:1], AF.Relu,
                                 bias=bcol[:, 2:3], scale=1.0 / FREE)
            nc.tensor.matmul(hp_[:, 128:129], whd[:, 128:256], h1[:],
                             start=True, stop=True)
            h2 = small.tile([128, 1], F32, tag=f"h2{g}")
            nc.scalar.activation(h2[:], hp_[:, 128:129], AF.Relu,
                                 bias=bcol[:, 3:4])
            nc.tensor.matmul(hp_[0:1, 256:257], whd[:, 256:257], h2[:],
                             start=True, stop=True)
            oval = small.tile([1, 1], F32, tag=f"ov{g}")
            nc.scalar.activation(oval[:], hp_[0:1, 256:257], AF.Identity,
                                 bias=bh3[:])
            nc.sync.dma_start(out_d[g:g + 1, :], oval[:])

        # ---- schedule ----
        # L0 half-interleaved across graphs so the second seljm half can
        # still be in flight; L1/L2 layer-interleaved.
        bss = {}
        for g in range(GPC):
            xi_xj(g, 0)
            bst = spool.tile([128, 512], F32, tag="sp", name=f"bs{g}_0")
            bss[g] = [bst[:, 0:128], 0]
        msg_half(0, 0, 0, bss[0])
        msg_half(1, 0, 0, bss[1])
        msg_half(0, 0, 1, bss[0])
        msg_half(1, 0, 1, bss[1])
        aggs = [finish_agg(g, 0, bss[g]) for g in range(GPC)]
        for g in range(GPC):
            node_update(g, 0, aggs[g])
        for l in (1, 2):
            for g in range(GPC):
                xi_xj(g, l)
                if l < 2:
                    bst = spool.tile([128, 512], F32, tag="sp",
                                     name=f"bs{g}_{l}")
                    bss[g] = [bst[:, 0:128], 0]
                    msg_half(g, l, 0, bss[g])
                    msg_half(g, l, 1, bss[g])
                else:
                    msg_half(g, l, 0, None)
                    msg_half(g, l, 1, None)
            if l < 2:
                aggs = [finish_agg(g, l, bss[g]) for g in range(GPC)]
                for g in range(GPC):
                    node_update(g, l, aggs[g])
        for g in range(GPC):
            head(g)

        stack.close()
    nc.finalize()
    return nc


def _f8(x):
    return np.asarray(x, dtype=np.float32).astype(NPF8)


def prep_core_inputs(inputs, core):
    """Host-side layout/dtype prep for one core's GPC graphs."""
    gs = slice(core * GPC, (core + 1) * GPC)
    A = np.asarray(inputs["edge_index"][gs], np.float32)        # [GPC,N,N]
    x = np.asarray(inputs["x"][gs], np.float32)                 # [GPC,N,DN0]
    ea = np.asarray(inputs["edge_attr"][gs], np.float32)        # [GPC,N,N,DE0]
    w = {k: np.asarray(inputs[k], np.float32) for k in WEIGHT_NAMES}

    f = np.arange(FREE)
    c1f, tf, pf = f // 2048, (f // 128) % 16, f % 128
    jf = 8 * tf + c1f

    seljm = (np.arange(128)[:, None] == jf[None, :]).astype(NPF8)
    marow = np.zeros((GPC, FREE), NPF8)
    for g in range(GPC):
        marow[g] = (4.0 * (1.0 - A[g][pf, jf])).astype(NPF8)

    # stationary arena bytes: xt | wbf | brep | fp8 consts | seli_src
    stb = np.zeros((128, S_DYN[1] - S_XT), np.uint8)
    o = -S_XT

    def put(off, arr):                         # place raw bytes at col offset
        bb = arr.view(np.uint8).reshape(arr.shape[0], -1)
        stb[0:arr.shape[0], o + off:o + off + bb.shape[1]] = bb

    xt = np.zeros((64, 256), NPBF)
    for g in range(GPC):
        xt[:, g * 128:(g + 1) * 128] = x[g].T.astype(NPBF)
    put(S_XT, np.ascontiguousarray(xt))

    wbf = np.zeros((128, 1280), NPBF)
    for l in range(2):
        Dn = DN0 if l == 0 else DH
        wm, wn = w[f"w_msg_{l}"], w[f"w_node_{l}"]
        wbf[0:Dn, l * 384:l * 384 + 128] = wm[0:Dn].astype(NPBF)
        wbf[0:Dn, l * 384 + 128:l * 384 + 256] = wm[Dn:2 * Dn].astype(NPBF)
        wbf[0:Dn, l * 384 + 256:l * 384 + 384] = wn[0:Dn].astype(NPBF)
        wbf[0:DH, 1024 + l * 128:1024 + (l + 1) * 128] = wn[Dn:].astype(NPBF)
    wbf[0:DH, 768:896] = (0.5 * w["w_msg_2"][0:DH]).astype(NPBF)
    wbf[0:DH, 896:1024] = (0.5 * w["w_msg_2"][DH:2 * DH]).astype(NPBF)
    put(S_WBF, wbf)

    brep = np.zeros((128, 384), np.float32)
    for l in range(3):
        brep[:, l * 128:(l + 1) * 128] = w[f"b_msg_{l}"][None, :]
    put(S_BREP, brep)

    f8c = np.zeros((128, S_DYN[1] - S_WE0), NPF8)
    oc = -S_WE0
    We0 = w["w_msg_0"][2 * DN0:]
    for c1 in range(8):
        f8c[16 * c1:16 * (c1 + 1),
            oc + S_WE0 + c1 * 128:oc + S_WE0 + (c1 + 1) * 128] = _f8(We0)
    f8c[:, oc + S_WE1:oc + S_WE1 + 128] = _f8(w["w_msg_1"][2 * DH:])
    we2h = _f8(0.5 * w["w_msg_2"][2 * DH:])
    f8c[:, oc + S_WE2H2:oc + S_WE2H2 + 128] = we2h
    f8c[:, oc + S_WE2H2 + 128:oc + S_WE2H2 + 256] = we2h
    eye = np.eye(128, dtype=np.float32).astype(NPF8)
    f8c[:, oc + S_I2:oc + S_I2 + 128] = eye
    f8c[:, oc + S_I2 + 128:oc + S_I2 + 256] = eye
    for g in range(GPC):
        f8c[g, oc + S_MSTAT + g * 128:oc + S_MSTAT + (g + 1) * 128] = \
            np.asarray(-240.0, NPF8)
    seli = (np.arange(128)[:, None] == (np.arange(CH) % 128)[None, :])
    f8c[:, oc + S_SELI:oc + S_SELI + 512] = seli.astype(NPF8)
    put(S_WE0, f8c)

    # transposed fp8 edge features: [(j8,de), (g, t16, i)]
    e0t = np.ascontiguousarray(
        ea.reshape(GPC, N, 16, 8, DE0).transpose(3, 4, 0, 2, 1)
    ).reshape(128, GPC * 2048).astype(NPF8)

    pk2 = np.zeros((128, PK2_TOTAL), np.uint8)
    bcol = np.stack([w["b_node_0"], w["b_node_1"], w["b_h1"], w["b_h2"]],
                    axis=1).astype(np.float32)
    pk2[:, 0:16] = bcol.view(np.uint8)
    whd = np.zeros((128, 257), np.float32)
    whd[:, 0:128] = w["w_h1"]
    whd[:, 128:256] = w["w_h2"]
    whd[:, 256:257] = w["w_h3"]
    pk2[:, 16:1044] = whd.view(np.uint8)
    pk2[0, 1044:1048] = w["b_h3"].astype(np.float32).view(np.uint8)

    return {
        "stconst": stb.view(NPF8), "seljm": seljm, "marow": marow,
        "e0t": e0t, "pk2": pk2.view(NPF8),
    }


def kernel(**inputs):
    inputs = {k: np.asarray(v) for k, v in inputs.items()}
    if "nc" not in _CACHE:
        _CACHE["nc"] = build_nc()
    nc = _CACHE["nc"]

    in_maps = [prep_core_inputs(inputs, c) for c in range(NCORES)]

    from concourse.bass_utils import run_bass_kernel_spmd
    res = run_bass_kernel_spmd(nc, in_maps, list(range(NCORES)))
    out = np.concatenate([np.asarray(res.results[c]["out"]).reshape(-1)
                          for c in range(NCORES)])
    return out.astype(np.float32)


# revision 37
# speedup vs baseline: 1.0930x; 1.0315x over previous
"""Trainium2 Bass kernel for nn_Disc_edge2 (3-layer dense-graph GNN + MLP head).

Sharding: data-parallel over batch B=16 across 8 cores (2 graphs/core).

Per-graph msg layout: [do=128 partitions, f=16384] with f = c1*2048 + t*128 + p,
edge (i, j) -> p = i, j = 8*t + c1.

All heavy compute runs as fp8e4m3 DoubleRow matmuls (2 k-tiles per pass, 0.5
cycles/row). Per 512-col chunk, layers 0/1 need just TWO DoubleRow matmuls:
    DR1: (We   @ e-chunk)   + (xib @ seli)     e-term + xi broadcast
    DR2: (xjb  @ seljm)     + (mstat @ maskA)  xj broadcast + adjacency mask
The adjacency mask is folded into the PSUM accumulation as -960*(1-A[f]) so the
relu eviction zeroes non-edges for free; no tensor-tensor mask pass exists.
Layer 2 adds a third DR for the residual e-blend (msg0@We2' + msg1@We2', with
the 0.5 folded into We2') and accumulates the edge-mean readout via accum_out
on the eviction op; msg2 is never materialized.

The j-aggregation runs on PE as accumulating [I|I] DoubleRows over j-block
pairs (block-sum in PSUM), interleaved into the chunk stream two pairs behind
the evictions; agg@Wa then happens in bf16 on the node-update matmul.

Evictions (PSUM->SBUF relu, the only remaining elementwise work) alternate
ACT / DVE in [128,1024] two-bank ops. GPSIMD cannot touch PSUM.

DMA issue costs ~1.3us of sequencer time per dma_start, so all constants are
byte-packed host-side into a handful of big transfers and sub-addressed with
bitcast views. The operand "arenas" are single SBUF tiles so DoubleRow k-tile
pairs can be addressed by inserting a [stride, 2] dim into the access
patterns (ISA steps are 16-bit, hence the second seli copy near graph 1's msg
regions).
"""

import sys

sys.path.insert(0, "/opt/trn_rl_repo")

import numpy as np
import ml_dtypes

import concourse.bass as bass
from concourse import bacc
import concourse.mybir as mybir
import concourse.tile as tile

F32 = mybir.dt.float32
BF16 = mybir.dt.bfloat16
F8 = mybir.dt.float8e4
AF = mybir.ActivationFunctionType
OP = mybir.AluOpType
DR = mybir.MatmulPerfMode.DoubleRow

NPF8 = ml_dtypes.float8_e4m3
NPBF = ml_dtypes.bfloat16

B, N, DN0, DE0, DH = 16, 128, 64, 16, 128
NCORES = 8
GPC = B // NCORES
FREE = N * N              # 16384
CH = 512
NCH = FREE // CH          # 32 chunks
NPAIR = NCH // 2          # 16 chunk-pairs ([128,1024] evictions)

# ---- moving arena (fp8) column offsets ----
O_SELJM = 0
O_MASKA = O_SELJM + FREE          # 16384
O_SELI = O_MASKA + FREE           # 32768
O_E0T = [O_SELI + CH, O_SELI + CH + 2048]        # per graph
O_MSG = [[O_E0T[1] + 2048, O_E0T[1] + 2048 + FREE],
         [O_E0T[1] + 2048 + 2 * FREE + CH, O_E0T[1] + 2048 + 3 * FREE + CH]]
O_SELI_B = O_MSG[0][1] + FREE     # second seli copy, just before msg0_g1
O_SCR = O_MSG[1][1] + FREE        # 2 x 1024 scratch (ACT/DVE)
MV_TOTAL = O_SCR + 2 * 1024

# ---- stationary arena (fp8 tile, byte-addressed constants) ----
# [dyn g0 | CRIT (first DMA) | REST (second DMA) | dyn g1]
S_DYN = [0, None]
S_XT = 768                        # [64,128] bf16 per graph, side by side
S_WI0 = S_XT + 512                # L0 Wi|Wj copy (bf16 [64,256])
S_BREP0 = S_WI0 + 512             # L0 b_msg replicated (f32 [128,128])
S_WE0 = S_BREP0 + 512             # 8 x 128 block-diag variants
S_MSTAT = S_WE0 + 1024            # per graph 128
S_ZERO = S_MSTAT + 256
S_SELI = S_ZERO + 128             # seli content, copied into mva
S_I2 = S_SELI + 512               # [I | I] for j-block-sum DoubleRows
S_CRIT_END = S_I2 + 256
S_WBF = S_CRIT_END                # [128,1280] bf16
S_BREP = S_WBF + 2560             # [128,384] f32 (col 0 unused)
S_WE1 = S_BREP + 1536
S_WE2H2 = S_WE1 + 128             # [0.5*We2 | 0.5*We2]
S_REST_END = S_WE2H2 + 256
S_DYN[1] = S_REST_END
ST_TOTAL = S_DYN[1] + 768
# pk2: bcol(16B) | whd(1028B) | bh3(4B)
PK2_TOTAL = 1048

WEIGHT_NAMES = [
    "w_msg_0", "b_msg_0", "w_node_0", "b_node_0",
    "w_msg_1", "b_msg_1", "w_node_1", "b_node_1",
    "w_msg_2", "b_msg_2", "w_node_2", "b_node_2",
    "w_h1", "b_h1", "w_h2", "b_h2", "w_h3", "b_h3",
]

_CACHE = {}


def _pair(ap_a, ap_b):
    """AP reading ap_a as k-tile slot 0 and ap_b as slot 1 (inserted dim)."""
    c = ap_a.copy()
    c.ap.insert(1, [ap_b.offset - ap_a.offset, 2])
    return c


def build_nc():
    nc = bacc.Bacc()

    crit_d = nc.declare_dram_parameter("crit", [128, S_CRIT_END - S_XT], F8,
                                       isOutput=False)
    rest_d = nc.declare_dram_parameter("rest", [128, S_REST_END - S_WBF], F8,
                                       isOutput=False)
    sj_d = nc.declare_dram_parameter("seljm", [128, FREE], F8, isOutput=False)
    marow_d = nc.declare_dram_parameter("marow", [GPC, FREE], F8, isOutput=False)
    e0t_d = nc.declare_dram_parameter("e0t", [128, GPC * 2048], F8,
                                      isOutput=False)
    pk2_d = nc.declare_dram_parameter("pk2", [128, PK2_TOTAL], F8,
                                      isOutput=False)
    out_d = nc.declare_dram_parameter("out", [GPC, 1], F32, isOutput=True)

    with tile.TileContext(nc) as tc:
        import contextlib
        stack = contextlib.ExitStack()
        gbuf = stack.enter_context(tc.tile_pool(name="gbuf", bufs=1))
        small = stack.enter_context(tc.tile_pool(name="small", bufs=2))
        zpool = stack.enter_context(tc.tile_pool(name="zp", bufs=3, space="PSUM"))
        spool = stack.enter_context(tc.tile_pool(name="sp", bufs=2, space="PSUM"))

        mva = gbuf.tile([128, MV_TOTAL], F8, tag="mva")
        sta = gbuf.tile([128, ST_TOTAL], F8, tag="sta")
        pk2 = gbuf.tile([128, PK2_TOTAL], F8, tag="pk2")
        hsum = [gbuf.tile([128, NPAIR], F32, tag=f"hs{g}", name=f"hs{g}")
                for g in range(GPC)]

        # maskA zero-fill on the otherwise-idle Pool engine (f32 view)
        nc.gpsimd.memset(mva[:, O_MASKA:O_MASKA + FREE].bitcast(F32), 0.0)

        # ---- sync-queue DMAs, finely split in consumption order ----
        nc.sync.dma_start(sta[:, S_XT:S_CRIT_END], crit_d[:, :])
        nc.sync.dma_start(mva[:, O_SELJM:O_SELJM + 4096], sj_d[:, 0:4096])
        nc.sync.dma_start(mva[0:GPC, O_MASKA:O_MASKA + FREE], marow_d[:, :])
        nc.sync.dma_start(mva[:, O_E0T[0]:O_E0T[0] + 2048],
                          e0t_d[:, 0:2048])
        nc.sync.dma_start(mva[:, O_SELJM + 4096:O_SELJM + 8192],
                          sj_d[:, 4096:8192])
        nc.sync.dma_start(mva[:, O_E0T[1]:O_E0T[1] + 2048],
                          e0t_d[:, 2048:4096])
        nc.sync.dma_start(sta[:, S_WBF:S_REST_END], rest_d[:, :])
        nc.sync.dma_start(mva[:, O_SELJM + 8192:O_SELJM + FREE],
                          sj_d[:, 8192:FREE])
        nc.sync.dma_start(pk2[:], pk2_d[:, :])

        # seli copies into the moving arena (Pool, SBUF->SBUF)
        nc.gpsimd.tensor_copy(mva[:, O_SELI:O_SELI + CH],
                              sta[:, S_SELI:S_SELI + 512])
        nc.gpsimd.tensor_copy(mva[:, O_SELI_B:O_SELI_B + CH],
                              sta[:, S_SELI:S_SELI + 512])

        def wslice(p0, p1, c0, c1):            # bf16 view of wbf
            return sta[p0:p1, S_WBF + 2 * c0:S_WBF + 2 * c1].bitcast(BF16)

        def brep_ap(l):
            return sta[:, S_BREP + 512 * l:S_BREP + 512 * (l + 1)].bitcast(F32)

        bcol = pk2[:, 0:16].bitcast(F32)       # [128,4]
        whd = pk2[:, 16:1044].bitcast(F32)     # [128,257]
        bh3 = pk2[0:1, 1044:1048].bitcast(F32)

        seli_ap = mva[:, O_SELI:O_SELI + CH]
        xTs = [sta[0:DN0, S_XT:S_XT + 256].bitcast(BF16),
               sta[0:DN0, S_XT + 256:S_XT + 512].bitcast(BF16)]

        def xi_xj(g, l):
            """xi' = x@Wi + b, xj' = x@Wj as fp8 stationaries in the arena."""
            xT = xTs[g]
            K = DN0 if l == 0 else DH
            wcol = l * 384 if l < 2 else 768
            if l == 0:
                wi = sta[0:K, S_WI0:S_WI0 + 512].bitcast(BF16)[:, 0:128]
                wj = sta[0:K, S_WI0:S_WI0 + 512].bitcast(BF16)[:, 128:256]
                br = sta[:, S_BREP0:S_BREP0 + 512].bitcast(F32)
            else:
                wi = wslice(0, K, wcol, wcol + 128)
                wj = wslice(0, K, wcol + 128, wcol + 256)
                br = brep_ap(l)
            ps = zpool.tile([128, 1024], F32, tag="z")
            nc.tensor.matmul(ps[:, 0:128], xT, wi, start=True, stop=True)
            nc.tensor.matmul(ps[:, 128:256], xT, wj, start=True, stop=True)
            dyn = S_DYN[g]
            nc.vector.tensor_tensor(sta[:, dyn + l * 256:dyn + l * 256 + 128],
                                    ps[:, 0:128], br, op=OP.add)
            nc.scalar.copy(sta[:, dyn + l * 256 + 128:dyn + l * 256 + 256],
                           ps[:, 128:256])

        i2_ap = sta[:, S_I2:S_I2 + 256].rearrange("p (two m) -> p two m", two=2)
        we2_ap = sta[:, S_WE2H2:S_WE2H2 + 256].rearrange(
            "p (two m) -> p two m", two=2)

        def msg_half(g, l, half, bs):
            """Chunk-pair DRs + evictions for half a layer; block-sum DRs
            (bs = [psum_ap, next_block]) trail the evictions by 2 pairs."""
            dyn = S_DYN[g]
            xib = sta[:, dyn + l * 256:dyn + l * 256 + 128]
            xjb = sta[:, dyn + l * 256 + 128:dyn + l * 256 + 256]
            mstat = sta[:, S_MSTAT + g * 128:S_MSTAT + (g + 1) * 128]
            zero = sta[:, S_ZERO:S_ZERO + 128]
            mo = O_MSG[g][l] if l < 2 else None

            def drain_bs(upto):
                if bs is None:
                    return
                while bs[1] < upto:
                    b2 = bs[1]
                    mb = mva[:, mo + b2 * 256:mo + (b2 + 1) * 256].rearrange(
                        "p (two m) -> p two m", two=2)
                    nc.tensor.matmul(bs[0], i2_ap, mb, start=(b2 == 0),
                                     stop=(b2 == 63), perf_mode=DR)
                    bs[1] += 1

            for cp in range(half * (NPAIR // 2), (half + 1) * (NPAIR // 2)):
                z = zpool.tile([128, 1024], F32, tag="z")
                for h in range(2):
                    k = 2 * cp + h
                    zz = z[:, h * 512:(h + 1) * 512]
                    sjm = mva[:, O_SELJM + k * CH:O_SELJM + (k + 1) * CH]
                    mka = mva[:, O_MASKA + k * CH:O_MASKA + (k + 1) * CH]
                    if l == 0:
                        c1, t4 = divmod(k, 4)
                        emv = mva[:, O_E0T[g] + t4 * CH:O_E0T[g] + (t4 + 1) * CH]
                        est = sta[:, S_WE0 + c1 * 128:S_WE0 + (c1 + 1) * 128]
                        nc.tensor.matmul(zz, _pair(est, xib), _pair(emv, seli_ap),
                                         start=True, stop=False, perf_mode=DR)
                    elif l == 1:
                        po = O_MSG[g][0]
                        emv = mva[:, po + k * CH:po + (k + 1) * CH]
                        est = sta[:, S_WE1:S_WE1 + 128]
                        sel = seli_ap if g == 0 else \
                            mva[:, O_SELI_B:O_SELI_B + CH]
                        nc.tensor.matmul(zz, _pair(est, xib), _pair(emv, sel),
                                         start=True, stop=False, perf_mode=DR)
                    else:
                        p0, p1 = O_MSG[g][0], O_MSG[g][1]
                        nc.tensor.matmul(
                            zz, we2_ap,
                            _pair(mva[:, p0 + k * CH:p0 + (k + 1) * CH],
                                  mva[:, p1 + k * CH:p1 + (k + 1) * CH]),
                            start=True, stop=False, perf_mode=DR)
                        nc.tensor.matmul(zz, _pair(xib, xjb),
                                         _pair(seli_ap, sjm),
                                         start=False, stop=False, perf_mode=DR)
                        nc.tensor.matmul(zz, _pair(mstat, zero),
                                         _pair(mka, sjm),
                                         start=False, stop=True, perf_mode=DR)
                    if l < 2:
                        nc.tensor.matmul(zz, _pair(xjb, mstat), _pair(sjm, mka),
                                         start=False, stop=True, perf_mode=DR)
                # block-sum DRs trail the evictions by one pair
                drain_bs(max(0, (cp - 1) * 4))
                # evict the chunk-pair: relu (+mask already in PSUM)
                on_act = cp % 2 == 0 or (cp == 15 and (g + l) % 2 == 0)
                if l < 2:
                    dst = mva[:, mo + cp * 1024:mo + (cp + 1) * 1024]
                    if on_act:
                        nc.scalar.activation(dst, z[:], AF.Relu)
                    else:
                        nc.vector.tensor_scalar(dst, z[:], 0.0, None, OP.max)
                else:
                    acc = hsum[g][:, cp:cp + 1]
                    if on_act:
                        scr = mva[:, O_SCR:O_SCR + 1024]
                        nc.scalar.activation(scr, z[:], AF.Relu, accum_out=acc)
                    else:
                        scr = mva[:, O_SCR + 1024:O_SCR + 2048]
                        nc.vector.tensor_scalar(scr, z[:], 0.0, None, OP.max,
                                                op1=OP.add, accum_out=acc)

        def finish_agg(g, l, bs):
            """Drain remaining block-sum DRs, evict agg to bf16 SBUF."""
            dyn = S_DYN[g]
            mo = O_MSG[g][l]
            while bs[1] < 64:
                b2 = bs[1]
                mb = mva[:, mo + b2 * 256:mo + (b2 + 1) * 256].rearrange(
                    "p (two m) -> p two m", two=2)
                nc.tensor.matmul(bs[0], i2_ap, mb, start=(b2 == 0),
                                 stop=(b2 == 63), perf_mode=DR)
                bs[1] += 1
            aggS = small.tile([128, 128], BF16, tag=f"agg{g}_{l}")
            nc.vector.tensor_copy(aggS[:], bs[0])
            return aggS

        def node_update(g, l, aggS):
            """x_new^T = relu(Wx^T x^T + Wa^T agg^T + b) with bf16 Wa."""
            xT = xTs[g]
            K = DN0 if l == 0 else DH
            wcol = l * 384 + 256
            xnt = spool.tile([128, 512], F32, tag="sp", name=f"xn{g}_{l}")
            xn = xnt[:, 0:128]
            nc.tensor.matmul(xn, wslice(0, K, wcol, wcol + 128), xT,
                             start=True, stop=False)
            nc.tensor.matmul(xn, wslice(0, DH, 1024 + l * 128,
                                        1024 + (l + 1) * 128),
                             aggS[:], start=False, stop=True)
            xnT = small.tile([128, 128], BF16, tag=f"xnT{g}_{l}")
            nc.scalar.activation(xnT[:], xn, AF.Relu, bias=bcol[:, l:l + 1])
            if l == 1:
                xbl = small.tile([128, 128], BF16, tag=f"xbl{g}")
                nc.vector.tensor_add(xbl[:], xnT[:], xTs[g])
                xTs[g] = xbl[:]
            else:
                xTs[g] = xnT[:]

        def head(g):
            hpre = small.tile([128, 1], F32, tag=f"hp{g}")
            nc.vector.tensor_reduce(hpre[:], hsum[g][:], axis=mybir.AxisListType.X,
                                    op=OP.add)
            hp_ = spool.tile([128, 512], F32, tag="sp")
            nc.tensor.matmul(hp_[:, 0:1], whd[:, 0:128], hpre[:],
                             start=True, stop=True)
            h1 = small.tile([128, 1], F32, tag=f"h1{g}")
            nc.scalar.activation(h1[:], hp_[:, 0:1], AF.Relu,
                                 bias=bcol[:, 2:3], scale=1.0 / FREE)
            nc.tensor.matmul(hp_[:, 128:129], whd[:, 128:256], h1[:],
                             start=True, stop=True)
            h2 = small.tile([128, 1], F32, tag=f"h2{g}")
            nc.scalar.activation(h2[:], hp_[:, 128:129], AF.Relu,
                                 bias=bcol[:, 3:4])
            nc.tensor.matmul(hp_[0:1, 256:257], whd[:, 256:257], h2[:],
                             start=True, stop=True)
            oval = small.tile([1, 1], F32, tag=f"ov{g}")
            nc.scalar.activation(oval[:], hp_[0:1, 256:257], AF.Identity,
                                 bias=bh3[:])
            nc.sync.dma_start(out_d[g:g + 1, :], oval[:])

        # ---- schedule ----
        # L0 half-interleaved across graphs so the second seljm half can
        # still be in flight; L1/L2 layer-interleaved.
        bss = {}
        for g in range(GPC):
            xi_xj(g, 0)
            bst = spool.tile([128, 512], F32, tag="sp", name=f"bs{g}_0")
            bss[g] = [bst[:, 0:128], 0]
        msg_half(0, 0, 0, bss[0])
        msg_half(1, 0, 0, bss[1])
        msg_half(0, 0, 1, bss[0])
        msg_half(1, 0, 1, bss[1])
        aggs = [finish_agg(g, 0, bss[g]) for g in range(GPC)]
        for g in range(GPC):
            node_update(g, 0, aggs[g])
        for l in (1, 2):
            for g in range(GPC):
                xi_xj(g, l)
                if l < 2:
                    bst = spool.tile([128, 512], F32, tag="sp",
                                     name=f"bs{g}_{l}")
                    bss[g] = [bst[:, 0:128], 0]
                    msg_half(g, l, 0, bss[g])
                    msg_half(g, l, 1, bss[g])
                else:
                    msg_half(g, l, 0, None)
                    msg_half(g, l, 1, None)
            if l < 2:
                aggs = [finish_agg(g, l, bss[g]) for g in range(GPC)]
                for g in range(GPC):
                    node_update(g, l, aggs[g])
        for g in range(GPC):
            head(g)

        stack.close()
    nc.finalize()
    return nc


def _f8(x):
    return np.asarray(x, dtype=np.float32).astype(NPF8)


def prep_core_inputs(inputs, core):
    """Host-side layout/dtype prep for one core's GPC graphs."""
    gs = slice(core * GPC, (core + 1) * GPC)
    A = np.asarray(inputs["edge_index"][gs], np.float32)        # [GPC,N,N]
    x = np.asarray(inputs["x"][gs], np.float32)                 # [GPC,N,DN0]
    ea = np.asarray(inputs["edge_attr"][gs], np.float32)        # [GPC,N,N,DE0]
    w = {k: np.asarray(inputs[k], np.float32) for k in WEIGHT_NAMES}

    f = np.arange(FREE)
    c1f, tf, pf = f // 2048, (f // 128) % 16, f % 128
    jf = 8 * tf + c1f

    seljm = (np.arange(128)[:, None] == jf[None, :]).astype(NPF8)
    marow = np.zeros((GPC, FREE), NPF8)
    for g in range(GPC):
        marow[g] = (4.0 * (1.0 - A[g][pf, jf])).astype(NPF8)

    # stationary arena bytes, crit block then rest block
    crit = np.zeros((128, S_CRIT_END - S_XT), np.uint8)
    rest = np.zeros((128, S_REST_END - S_WBF), np.uint8)

    def put(dst, base, off, arr):
        bb = arr.view(np.uint8).reshape(arr.shape[0], -1)
        dst[0:arr.shape[0], off - base:off - base + bb.shape[1]] = bb

    xt = np.zeros((64, 256), NPBF)
    for g in range(GPC):
        xt[:, g * 128:(g + 1) * 128] = x[g].T.astype(NPBF)
    put(crit, S_XT, S_XT, np.ascontiguousarray(xt))

    wm0 = w["w_msg_0"]
    wi0 = np.concatenate([wm0[0:DN0], wm0[DN0:2 * DN0]], axis=1).astype(NPBF)
    put(crit, S_XT, S_WI0, np.ascontiguousarray(wi0))          # [64,256] bf16
    put(crit, S_XT, S_BREP0,
        np.ascontiguousarray(np.repeat(w["b_msg_0"][None, :], 128, 0)))

    f8c = np.zeros((128, S_CRIT_END - S_WE0), NPF8)
    oc = -S_WE0
    We0 = wm0[2 * DN0:]
    for c1 in range(8):
        f8c[16 * c1:16 * (c1 + 1),
            oc + S_WE0 + c1 * 128:oc + S_WE0 + (c1 + 1) * 128] = _f8(We0)
    for g in range(GPC):
        f8c[g, oc + S_MSTAT + g * 128:oc + S_MSTAT + (g + 1) * 128] = \
            np.asarray(-240.0, NPF8)
    seli = (np.arange(128)[:, None] == (np.arange(CH) % 128)[None, :])
    f8c[:, oc + S_SELI:oc + S_SELI + 512] = seli.astype(NPF8)
    eye = np.eye(128, dtype=np.float32).astype(NPF8)
    f8c[:, oc + S_I2:oc + S_I2 + 128] = eye
    f8c[:, oc + S_I2 + 128:oc + S_I2 + 256] = eye
    put(crit, S_XT, S_WE0, f8c)

    wbf = np.zeros((128, 1280), NPBF)
    for l in range(2):
        Dn = DN0 if l == 0 else DH
        wm, wn = w[f"w_msg_{l}"], w[f"w_node_{l}"]
        wbf[0:Dn, l * 384:l * 384 + 128] = wm[0:Dn].astype(NPBF)
        wbf[0:Dn, l * 384 + 128:l * 384 + 256] = wm[Dn:2 * Dn].astype(NPBF)
        wbf[0:Dn, l * 384 + 256:l * 384 + 384] = wn[0:Dn].astype(NPBF)
        wbf[0:DH, 1024 + l * 128:1024 + (l + 1) * 128] = wn[Dn:].astype(NPBF)
    wbf[0:DH, 768:896] = (0.5 * w["w_msg_2"][0:DH]).astype(NPBF)
    wbf[0:DH, 896:1024] = (0.5 * w["w_msg_2"][DH:2 * DH]).astype(NPBF)
    put(rest, S_WBF, S_WBF, wbf)

    brep = np.zeros((128, 384), np.float32)
    for l in range(3):
        brep[:, l * 128:(l + 1) * 128] = w[f"b_msg_{l}"][None, :]
    put(rest, S_WBF, S_BREP, brep)

    f8r = np.zeros((128, S_REST_END - S_WE1), NPF8)
    orr = -S_WE1
    f8r[:, orr + S_WE1:orr + S_WE1 + 128] = _f8(w["w_msg_1"][2 * DH:])
    we2h = _f8(0.5 * w["w_msg_2"][2 * DH:])
    f8r[:, orr + S_WE2H2:orr + S_WE2H2 + 128] = we2h
    f8r[:, orr + S_WE2H2 + 128:orr + S_WE2H2 + 256] = we2h
    put(rest, S_WBF, S_WE1, f8r)

    # transposed fp8 edge features: [(j8,de), (g, t16, i)]
    e0t = np.ascontiguousarray(
        ea.reshape(GPC, N, 16, 8, DE0).transpose(3, 4, 0, 2, 1)
    ).reshape(128, GPC * 2048).astype(NPF8)

    pk2 = np.zeros((128, PK2_TOTAL), np.uint8)
    bcol = np.stack([w["b_node_0"], w["b_node_1"], w["b_h1"], w["b_h2"]],
                    axis=1).astype(np.float32)
    pk2[:, 0:16] = bcol.view(np.uint8)
    whd = np.zeros((128, 257), np.float32)
    whd[:, 0:128] = w["w_h1"]
    whd[:, 128:256] = w["w_h2"]
    whd[:, 256:257] = w["w_h3"]
    pk2[:, 16:1044] = whd.view(np.uint8)
    pk2[0, 1044:1048] = w["b_h3"].astype(np.float32).view(np.uint8)

    return {
        "crit": crit.view(NPF8), "rest": rest.view(NPF8), "seljm": seljm,
        "marow": marow, "e0t": e0t, "pk2": pk2.view(NPF8),
    }


def kernel(**inputs):
    inputs = {k: np.asarray(v) for k, v in inputs.items()}
    if "nc" not in _CACHE:
        _CACHE["nc"] = build_nc()
    nc = _CACHE["nc"]

    in_maps = [prep_core_inputs(inputs, c) for c in range(NCORES)]

    from concourse.bass_utils import run_bass_kernel_spmd
    res = run_bass_kernel_spmd(nc, in_maps, list(range(NCORES)))
    out = np.concatenate([np.asarray(res.results[c]["out"]).reshape(-1)
                          for c in range(NCORES)])
    return out.astype(np.float32)
